# revision 29
# baseline (speedup 1.0000x reference)
"""Fused GEMM + bias + residual + AvgPool2d(2) + global-mean normalize, 8-core SPMD.

Reference computation (B=8192, IN_F=1024, OUT_F=4096, S=64, K=2):
    out_lin = x @ W.T + bias + y                  # (B, 4096)
    pooled  = avgpool2x2(out_lin.reshape(B,64,64))# (B, 32, 32)
    out     = pooled / pooled.mean()              # (B, 1, 32, 32)

Algebraic folds (exact):
  * The 2x2 avg-pool is linear -> folds into weight/bias/residual:
        pooled_raw[b, m] = x[b] . Wsum[m] + bias_sum[m] + y_sum[b, m]
    with m = 32*i + j pooling OUT_F rows {128i+64r+2j+s : r,s in {0,1}}.
    GEMM N-dim shrinks 4096 -> 1024; the (B, 4096) intermediate is never
    materialized.
  * The 1/4 pool factor cancels against the global mean:
        out = pooled_raw * (B*1024 / sum_global(pooled_raw))
  * The global sum decomposes over raw inputs:
        local_sum = xsum . wcolsum + BL * bias_tot + ytot

Host staging (sharding/layout only): inputs are cast to bf16 and laid out
in SBUF-tile-major order on the host -- x and W pre-transposed to [K, *],
and the 4096-wide pooled axes of W and y permuted per tile to (q=rs, m)
order, so the on-device 4-tap pooling is three fully-contiguous
tensor_adds per tile (split across Vector and GpSimd).  Per-core HBM
traffic is 18 MiB in + 4 MiB out, all on HWDGE rings as 1 MiB tile
loads, and the kernel needs no on-chip transposes.

Cross-core scalar sum: either one collective AllReduce (USE_REMOTE=False)
or a low-latency XOR all-to-all built from 7 single-destination
remote_dma_broadcast calls (USE_REMOTE=True): slot d of the receive
buffer gets the partial sum of core (me XOR d), so one compile-time SPMD
program needs no core ids.

Sharding: batch B split 8 ways (1024 rows/core); weight + bias replicated.
"""

import numpy as np
import ml_dtypes

import concourse.bass as bass
import concourse.mybir as mybir
import concourse.tile as tile
from concourse import bacc
from concourse.bass import ts
from concourse.bass_utils import run_bass_kernel_spmd
from concourse.masks import make_identity

N_CORES = 8
B = 8192
BL = B // N_CORES          # 1024 batch rows per core
KF = 1024                  # IN_F (contraction)
NF = 4096                  # OUT_F
M = 1024                   # pooled features (32*32)
TOT = float(B * M)         # elements in the global mean
F32 = mybir.dt.float32
BF16 = mybir.dt.bfloat16
ADD = mybir.AluOpType.add
MULT = mybir.AluOpType.mult

USE_REMOTE = False          # scalar exchange: remote_dma vs collective

_CACHE = {}


def _pool_perm():
    """n-axis permutation: q=(r,s)-major, pooled-feature-minor order."""
    m = np.arange(M)
    i, j = m // 32, m % 32
    cols = []
    for r in (0, 1):
        for s in (0, 1):
            cols.append(128 * i + 64 * r + 2 * j + s)
    return np.concatenate(cols)


def build_nc(use_remote=USE_REMOTE):
    nc = bacc.Bacc("TRN2", target_bir_lowering=False, debug=False,
                   num_devices=N_CORES)
    # host-staged bf16 layouts: xt [k-part, kt, b]; wt/y [part, tile, q, m]
    xt = nc.dram_tensor("xt", [128, 8, BL], BF16, kind="ExternalInput").ap()
    wt = nc.dram_tensor("wt", [128, 8, 4, M], BF16,
                        kind="ExternalInput").ap()
    yd = nc.dram_tensor("y", [128, 8, 4, M], BF16, kind="ExternalInput").ap()
    bd = nc.dram_tensor("b", [1, NF], BF16, kind="ExternalInput").ap()
    out = nc.dram_tensor("out", [128, 8, M], BF16,
                         kind="ExternalOutput").ap()

    ring = [nc.sync, nc.scalar]

    # raw SBUF tensors shared between the Tile region and the raw region
    orb_r = [nc.alloc_sbuf_tensor(f"orb{i}", [128, M], BF16).ap()
             for i in range(8)]
    loc128 = nc.alloc_sbuf_tensor("loc128", [128, 1], F32).ap()
    slots = nc.alloc_sbuf_tensor("slots", [128, 8], F32).ap()
    rsb = nc.alloc_sbuf_tensor("rsb", [128, 1], F32).ap()
    gsr = nc.alloc_sbuf_tensor("gsr", [128, 1], F32).ap()

    with tile.TileContext(nc) as tc:
        with (
            tc.tile_pool(name="consts", bufs=1) as consts,
            tc.tile_pool(name="xtp", bufs=1) as xtp,
            tc.tile_pool(name="wload", bufs=7) as wload,
            tc.tile_pool(name="yload", bufs=8) as yload,
            tc.tile_pool(name="prp", bufs=2) as prp,
            tc.tile_pool(name="wsp", bufs=1) as wsp,
            tc.tile_pool(name="statsp", bufs=1) as statsp,
            tc.tile_pool(name="outp", bufs=2) as outp,
            tc.tile_pool(name="psA", bufs=4, space="PSUM") as psA,
            tc.tile_pool(name="psB", bufs=4, space="PSUM") as psB,
            tc.tile_pool(name="dram", bufs=1, space="DRAM") as dram,
        ):
            # ---- constants ----
            ident_bf = consts.tile([128, 128], BF16)
            make_identity(nc, ident_bf)
            ones_row_bf = consts.tile([1, 128], BF16)
            nc.vector.memset(ones_row_bf, 1.0)
            ones_row_f = consts.tile([1, 128], F32)
            nc.vector.memset(ones_row_f, 1.0)
            ones_col = consts.tile([128, 1], F32)
            nc.vector.memset(ones_col, 1.0)
            ones_one = consts.tile([1, 1], F32)
            nc.vector.memset(ones_one, 1.0)

            # ---- bias: load, pool 4096 -> 1024 (raw order), totals ----
            bload = consts.tile([1, NF], BF16)
            nc.sync.dma_start(out=bload, in_=bd)
            blv = bload.rearrange("o (i r j s) -> o i r j s", r=2, j=32, s=2)
            bsum = consts.tile([1, 32, 32], F32)
            nc.vector.tensor_add(bsum, blv[:, :, 0, :, 0], blv[:, :, 0, :, 1])
            nc.vector.tensor_add(bsum, bsum, blv[:, :, 1, :, 0])
            nc.vector.tensor_add(bsum, bsum, blv[:, :, 1, :, 1])
            bsum_bf = consts.tile([1, M], BF16)
            nc.vector.tensor_copy(out=bsum_bf,
                                  in_=bsum.rearrange("o i j -> o (i j)"))

            # ---- dummy AllReduce at t~0 absorbs the collective entry
            # barrier + firmware wakeup, shrinking the real one's latency ----
            if not use_remote:
                warm = statsp.tile([1, 1], F32)
                nc.vector.memset(warm, 0.0)
                cc_win = dram.tile([1, 1], F32, name="cc_win")
                cc_wout = dram.tile([8, 1], F32, name="cc_wout")
                nc.sync.dma_start(out=cc_win, in_=warm)
                nc.gpsimd.collective_compute(
                    "AllGather", mybir.AluOpType.bypass,
                    replica_groups=[list(range(N_CORES))],
                    ins=[cc_win.opt()], outs=[cc_wout.opt()])

            # ---- x^T first on both rings, resident ----
            xts = xtp.tile([128, 8, BL], BF16)
            ring[0].dma_start(out=xts[:, 0:4, :], in_=xt[:, 0:4, :])
            ring[1].dma_start(out=xts[:, 4:8, :], in_=xt[:, 4:8, :])

            # ---- issue ALL tile loads upfront on dedicated buffers:
            # nothing backpressures the rings, stream runs at wire speed ----
            wls = []
            for g in range(8):
                wl = wload.tile([128, 4, M], BF16, tag="wl", name=f"wl{g}")
                ring[g % 2].dma_start(out=wl, in_=wt[:, g, :, :])
                wls.append(wl)
            yls = []
            for bt in range(8):
                yl = yload.tile([128, 4, M], BF16, tag="yl", name=f"yl{bt}")
                ring[bt % 2].dma_start(out=yl, in_=yd[:, bt, :, :])
                yls.append(yl)

            # ---- W pooling on vector (3 contiguous adds per tile) ----
            wsum_all = wsp.tile([128, 8, M], BF16)
            for g in range(8):
                wl = wls[g]
                p1 = prp.tile([128, M], BF16, tag="p1", name=f"wp1_{g}")
                p2 = prp.tile([128, M], BF16, tag="p2", name=f"wp2_{g}")
                nc.vector.tensor_add(p1, wl[:, 0, :], wl[:, 1, :])
                nc.vector.tensor_add(p2, wl[:, 2, :], wl[:, 3, :])
                nc.vector.tensor_add(wsum_all[:, g, :], p1, p2)

            # ---- y pooling; ys tiles overlay dead y-load slots ----
            ys_tiles = {}

            def ypool(bt):
                yl = yls[bt]
                q1 = prp.tile([128, M], BF16, tag="q1", name=f"yq1_{bt}")
                q2 = prp.tile([128, M], BF16, tag="q2", name=f"yq2_{bt}")
                # gpsimd (slow) helps only on mid tiles, never the tail
                e2 = nc.gpsimd if 1 <= bt <= 4 else nc.vector
                ysb = yload.tile([128, M], BF16, tag="yl", name=f"ys{bt}")
                ys_tiles[bt] = ysb
                nc.vector.tensor_add(q1, yl[:, 0, :], yl[:, 1, :])
                e2.tensor_add(q2, yl[:, 2, :], yl[:, 3, :])
                nc.vector.tensor_add(ysb, q1, q2)

            # ---- GEMM in two 4-bt groups, kb-major ----
            or_tiles = {}
            osum = statsp.tile([128, 8], F32)

            def gemm_matmuls(bts):
                mm = {}
                for bt in bts:
                    mm[bt] = [psA.tile([128, 512], F32, tag="mmA",
                                       name=f"mmA{bt}"),
                              psB.tile([128, 512], F32, tag="mmB",
                                       name=f"mmB{bt}")]
                for kb in range(8):
                    for bt in bts:
                        for mh in range(2):
                            nc.tensor.matmul(mm[bt][mh],
                                             xts[:, kb, ts(bt, 128)],
                                             wsum_all[:, kb, ts(mh, 512)],
                                             start=(kb == 0), stop=False)
                return mm

            def gemm_close(mm, bt):
                orb = orb_r[bt]
                or_tiles[bt] = orb
                for mh in range(2):
                    nc.tensor.matmul(mm[bt][mh], ones_row_bf,
                                     bsum_bf[:, ts(mh, 512)],
                                     start=False, stop=True)
                    # drain PSUM + add pooled-y on vector (free at y-tail)
                    nc.vector.tensor_add(orb[:, ts(mh, 512)], mm[bt][mh],
                                         ys_tiles[bt][:, ts(mh, 512)])
                nc.vector.reduce_sum(out=osum[:, bt:bt + 1], in_=orb,
                                     axis=mybir.AxisListType.X)

            mmA_ = gemm_matmuls([0, 1, 2, 3])
            ypool(0)
            ypool(1)
            for bt in (0, 1):
                gemm_close(mmA_, bt)
            ypool(2)
            ypool(3)
            for bt in (2, 3):
                gemm_close(mmA_, bt)
            mmB_ = gemm_matmuls([4, 5, 6, 7])
            ypool(4)
            ypool(5)
            for bt in (4, 5):
                gemm_close(mmB_, bt)
            ypool(6)
            ypool(7)
            for bt in (6, 7):
                gemm_close(mmB_, bt)

            # ---- local sum -> global sum exchange ----
            part = statsp.tile([128, 1], F32)
            nc.vector.reduce_sum(out=part, in_=osum,
                                 axis=mybir.AxisListType.X)
            ls_ps = psA.tile([1, 1], F32, tag="mmA", name="ls_ps")
            nc.tensor.matmul(ls_ps, part, ones_col, start=True, stop=True)
            ls2 = statsp.tile([1, 1], F32)
            nc.scalar.copy(out=ls2, in_=ls_ps)

            if use_remote:
                bc_ps = psB.tile([128, 1], F32, tag="mmB", name="bc_ps")
                nc.tensor.matmul(bc_ps, ones_row_f, ls2, start=True,
                                 stop=True)
                nc.scalar.copy(out=loc128, in_=bc_ps)
                nc.vector.tensor_copy(out=slots[:, 0:1], in_=loc128)
            else:
                cc_in = dram.tile([1, 1], F32)
                cc_out = dram.tile([8, 1], F32)
                nc.sync.dma_start(out=cc_in, in_=ls2)
                # AllGather of the 8 scalars: one ring pass (vs two for
                # AllReduce); sum + partition-broadcast done locally
                nc.gpsimd.collective_compute(
                    "AllGather", mybir.AluOpType.bypass,
                    replica_groups=[list(range(N_CORES))],
                    ins=[cc_in.opt()], outs=[cc_out.opt()])
                g8 = statsp.tile([1, 8], F32)
                nc.sync.dma_start(out=g8, in_=cc_out.rearrange("a o -> o a"))
                ls3 = statsp.tile([1, 1], F32)
                nc.vector.reduce_sum(out=ls3, in_=g8,
                                     axis=mybir.AxisListType.X)
                gb_ps = psB.tile([128, 1], F32, tag="mmB", name="gb_ps")
                nc.tensor.matmul(gb_ps, ones_row_f, ls3, start=True,
                                 stop=True)
                gsb = statsp.tile([128, 1], F32)
                nc.scalar.copy(out=gsb, in_=gb_ps)
                nc.vector.reciprocal(rsb, gsb)
                for bt in range(8):
                    ot = outp.tile([128, M], BF16)
                    nc.vector.tensor_scalar(out=ot, in0=or_tiles[bt],
                                            scalar1=rsb, scalar2=TOT,
                                            op0=MULT, op1=MULT)
                    ring[bt % 2].dma_start(out=out[:, bt, :], in_=ot)

    if use_remote:
        # ---- raw region (outside Tile): XOR all-to-all scalar exchange,
        # then scales + stores with manual semaphores.  The Tile scheduler
        # cannot model remote semaphore arrivals, hence the split. ----
        ps = nc.alloc_semaphore("xch_prep")
        rs = nc.alloc_semaphore("xch_recv")
        lsm = nc.alloc_semaphore("xch_sent")
        sv = nc.alloc_semaphore("xch_scaled")
        so = nc.alloc_semaphore("xch_stored")
        for d in range(1, 8):
            rdests = [None] * 8
            rdests[d] = (0, d)
            nc.gpsimd.remote_dma_broadcast(
                out_ap=slots[:, d:d + 1], in_ap=loc128,
                remote_sem=rs, local_sem=lsm,
                rdests=rdests).then_inc(ps, 1)
        nc.gpsimd.wait_ge(ps, 7)
        nc.gpsimd.trigger_dma(count=7)
        nc.vector.wait_ge(rs, 14)
        nc.vector.reduce_sum(out=gsr, in_=slots, axis=mybir.AxisListType.X)
        nc.vector.reciprocal(rsb, gsr)
        for bt in range(8):
            nc.vector.tensor_scalar(out=orb_r[bt], in0=orb_r[bt],
                                    scalar1=rsb, scalar2=TOT,
                                    op0=MULT, op1=MULT).then_inc(sv, 1)
        for bt in range(8):
            ring[bt % 2].wait_ge(sv, bt + 1)
            ring[bt % 2].dma_start(out=out[:, bt, :],
                                   in_=orb_r[bt]).then_inc(so, 16)
        nc.gpsimd.wait_ge(lsm, 112)
        nc.gpsimd.wait_ge(so, 128)
        nc.clear_and_free_semaphores([ps, rs, lsm, sv, so])

    nc.compile()
    return nc


def _stage_x(a):
    """x slice [BL, KF] -> transpose -> bf16 [128, 8, BL]."""
    t = a.T.astype(ml_dtypes.bfloat16).reshape(8, 128, BL)
    return np.ascontiguousarray(t.transpose(1, 0, 2))


def _stage_pooled(a, perm, trans):
    """[rows, 4096] (optionally transposed first) -> bf16 [128, T, 4, M]."""
    if trans:
        a = a.T
    a = a[:, perm]                                   # (q, m) order
    r = a.shape[0]
    t = a.astype(ml_dtypes.bfloat16).reshape(r // 128, 128, 4, M)
    return np.ascontiguousarray(t.transpose(1, 0, 2, 3))


def _run(inputs, trace=False):
    if "nc" not in _CACHE:
        _CACHE["nc"] = build_nc()
    nc = _CACHE["nc"]
    x = np.asarray(inputs["x"], dtype=np.float32)
    y = np.asarray(inputs["y"], dtype=np.float32)
    w = np.asarray(inputs["weight"], dtype=np.float32)
    b = np.asarray(inputs["bias"], dtype=np.float32).reshape(1, NF)
    b = b.astype(ml_dtypes.bfloat16)
    perm = _pool_perm()
    wt_host = _stage_pooled(w, perm, trans=True)     # [128, 8, 4, 1024]
    in_maps = [
        {"xt": _stage_x(x[c * BL:(c + 1) * BL]),
         "y": _stage_pooled(y[c * BL:(c + 1) * BL], perm, trans=False),
         "wt": wt_host, "b": b}
        for c in range(N_CORES)
    ]
    res = run_bass_kernel_spmd(nc, in_maps, core_ids=list(range(N_CORES)),
                               trace=trace)
    full = np.concatenate(
        [res.results[c]["out"].transpose(1, 0, 2).reshape(BL, M)
         for c in range(N_CORES)], axis=0)
    return full.astype(np.float32).reshape(B, 1, 32, 32), res


def kernel(**inputs) -> np.ndarray:
    out, _ = _run(inputs, trace=False)
    return out


# revision 30
# speedup vs baseline: 1.5145x; 1.5145x over previous
"""Fused GEMM + bias + residual + AvgPool2d(2) + global-mean normalize, 8-core SPMD.

Reference computation (B=8192, IN_F=1024, OUT_F=4096, S=64, K=2):
    out_lin = x @ W.T + bias + y                  # (B, 4096)
    pooled  = avgpool2x2(out_lin.reshape(B,64,64))# (B, 32, 32)
    out     = pooled / pooled.mean()              # (B, 1, 32, 32)

Algebraic folds (exact):
  * The 2x2 avg-pool is linear -> folds into weight/bias/residual:
        pooled_raw[b, m] = x[b] . Wsum[m] + bias_sum[m] + y_sum[b, m]
    with m = 32*i + j pooling OUT_F rows {128i+64r+2j+s : r,s in {0,1}}.
    GEMM N-dim shrinks 4096 -> 1024; the (B, 4096) intermediate is never
    materialized.
  * The 1/4 pool factor cancels against the global mean:
        out = pooled_raw * (B*1024 / sum_global(pooled_raw))
  * The global sum decomposes over raw inputs:
        local_sum = xsum . wcolsum + BL * bias_tot + ytot

Host staging (sharding/layout only): inputs are cast to bf16 and laid out
in SBUF-tile-major order on the host -- x and W pre-transposed to [K, *],
and the 4096-wide pooled axes of W and y permuted per tile to (q=rs, m)
order, so the on-device 4-tap pooling is three fully-contiguous
tensor_adds per tile (split across Vector and GpSimd).  Per-core HBM
traffic is 18 MiB in + 4 MiB out, all on HWDGE rings as 1 MiB tile
loads, and the kernel needs no on-chip transposes.

Cross-core scalar sum: either one collective AllReduce (USE_REMOTE=False)
or a low-latency XOR all-to-all built from 7 single-destination
remote_dma_broadcast calls (USE_REMOTE=True): slot d of the receive
buffer gets the partial sum of core (me XOR d), so one compile-time SPMD
program needs no core ids.

Sharding: batch B split 8 ways (1024 rows/core); weight + bias replicated.
"""

import numpy as np
import ml_dtypes

import concourse.bass as bass
import concourse.mybir as mybir
import concourse.tile as tile
from concourse import bacc
from concourse.bass import ts
from concourse.bass_utils import run_bass_kernel_spmd
from concourse.masks import make_identity

N_CORES = 8
B = 8192
BL = B // N_CORES          # 1024 batch rows per core
KF = 1024                  # IN_F (contraction)
NF = 4096                  # OUT_F
M = 1024                   # pooled features (32*32)
TOT = float(B * M)         # elements in the global mean
F32 = mybir.dt.float32
BF16 = mybir.dt.bfloat16
ADD = mybir.AluOpType.add
MULT = mybir.AluOpType.mult

USE_REMOTE = False          # scalar exchange: remote_dma vs collective

_CACHE = {}


def _pool_perm():
    """n-axis permutation: q=(r,s)-major, pooled-feature-minor order."""
    m = np.arange(M)
    i, j = m // 32, m % 32
    cols = []
    for r in (0, 1):
        for s in (0, 1):
            cols.append(128 * i + 64 * r + 2 * j + s)
    return np.concatenate(cols)


def build_nc(use_remote=USE_REMOTE):
    nc = bacc.Bacc("TRN2", target_bir_lowering=False, debug=False,
                   num_devices=N_CORES)
    # host-staged bf16 layouts: xt [k-part, kt, b]; wt/y [part, tile, q, m]
    xt = nc.dram_tensor("xt", [128, 8, BL], BF16, kind="ExternalInput").ap()
    wt = nc.dram_tensor("wt", [128, 8, 4, M], BF16,
                        kind="ExternalInput").ap()
    yd = nc.dram_tensor("y", [128, 8, 4, M], BF16, kind="ExternalInput").ap()
    bd = nc.dram_tensor("b", [1, NF], BF16, kind="ExternalInput").ap()
    out = nc.dram_tensor("out", [128, 8, M], BF16,
                         kind="ExternalOutput").ap()

    ring = [nc.sync, nc.scalar]

    # raw SBUF tensors shared between the Tile region and the raw region
    orb_r = [nc.alloc_sbuf_tensor(f"orb{i}", [128, M], BF16).ap()
             for i in range(8)]
    loc128 = nc.alloc_sbuf_tensor("loc128", [128, 1], F32).ap()
    slots = nc.alloc_sbuf_tensor("slots", [128, 8], F32).ap()
    rsb = nc.alloc_sbuf_tensor("rsb", [128, 1], F32).ap()
    gsr = nc.alloc_sbuf_tensor("gsr", [128, 1], F32).ap()

    with tile.TileContext(nc) as tc:
        with (
            tc.tile_pool(name="consts", bufs=1) as consts,
            tc.tile_pool(name="xtp", bufs=1) as xtp,
            tc.tile_pool(name="wload", bufs=7) as wload,
            tc.tile_pool(name="yload", bufs=8) as yload,
            tc.tile_pool(name="prp", bufs=2) as prp,
            tc.tile_pool(name="wsp", bufs=1) as wsp,
            tc.tile_pool(name="statsp", bufs=1) as statsp,
            tc.tile_pool(name="outp", bufs=4) as outp,
            tc.tile_pool(name="psA", bufs=4, space="PSUM") as psA,
            tc.tile_pool(name="psB", bufs=4, space="PSUM") as psB,
            tc.tile_pool(name="dram", bufs=1, space="DRAM") as dram,
        ):
            # ---- constants ----
            ident_bf = consts.tile([128, 128], BF16)
            make_identity(nc, ident_bf)
            ones_row_bf = consts.tile([1, 128], BF16)
            nc.vector.memset(ones_row_bf, 1.0)
            ones_row_f = consts.tile([1, 128], F32)
            nc.vector.memset(ones_row_f, 1.0)
            ones_col = consts.tile([128, 1], F32)
            nc.vector.memset(ones_col, 1.0)
            ones_one = consts.tile([1, 1], F32)
            nc.vector.memset(ones_one, 1.0)

            # ---- bias: load, pool 4096 -> 1024 (raw order), totals ----
            bload = consts.tile([1, NF], BF16)
            nc.sync.dma_start(out=bload, in_=bd)
            blv = bload.rearrange("o (i r j s) -> o i r j s", r=2, j=32, s=2)
            bsum = consts.tile([1, 32, 32], F32)
            nc.vector.tensor_add(bsum, blv[:, :, 0, :, 0], blv[:, :, 0, :, 1])
            nc.vector.tensor_add(bsum, bsum, blv[:, :, 1, :, 0])
            nc.vector.tensor_add(bsum, bsum, blv[:, :, 1, :, 1])
            bsum_bf = consts.tile([1, M], BF16)
            nc.vector.tensor_copy(out=bsum_bf,
                                  in_=bsum.rearrange("o i j -> o (i j)"))

            # ---- dummy AllReduce at t~0 absorbs the collective entry
            # barrier + firmware wakeup, shrinking the real one's latency ----
            if not use_remote:
                warm = statsp.tile([1, 1], F32)
                nc.vector.memset(warm, 0.0)
                cc_win = dram.tile([1, 1], F32, name="cc_win")
                cc_wout = dram.tile([8, 1], F32, name="cc_wout")
                nc.sync.dma_start(out=cc_win, in_=warm)
                nc.gpsimd.collective_compute(
                    "AllGather", mybir.AluOpType.bypass,
                    replica_groups=[list(range(N_CORES))],
                    ins=[cc_win.opt()], outs=[cc_wout.opt()])

            # ---- x^T first on both rings, resident ----
            xts = xtp.tile([128, 8, BL], BF16)
            ring[0].dma_start(out=xts[:, 0:4, :], in_=xt[:, 0:4, :])
            ring[1].dma_start(out=xts[:, 4:8, :], in_=xt[:, 4:8, :])

            # ---- issue ALL tile loads upfront on dedicated buffers:
            # nothing backpressures the rings, stream runs at wire speed ----
            wls = []
            for g in range(8):
                wl = wload.tile([128, 4, M], BF16, tag="wl", name=f"wl{g}")
                ring[g % 2].dma_start(out=wl, in_=wt[:, g, :, :])
                wls.append(wl)
            yls = []
            for bt in range(8):
                yl = yload.tile([128, 4, M], BF16, tag="yl", name=f"yl{bt}")
                ring[bt % 2].dma_start(out=yl, in_=yd[:, bt, :, :])
                yls.append(yl)

            # ---- W pooling on vector (3 contiguous adds per tile) ----
            wsum_all = wsp.tile([128, 8, M], BF16)
            for g in range(8):
                wl = wls[g]
                p1 = prp.tile([128, M], BF16, tag="p1", name=f"wp1_{g}")
                p2 = prp.tile([128, M], BF16, tag="p2", name=f"wp2_{g}")
                nc.vector.tensor_add(p1, wl[:, 0, :], wl[:, 1, :])
                nc.vector.tensor_add(p2, wl[:, 2, :], wl[:, 3, :])
                nc.vector.tensor_add(wsum_all[:, g, :], p1, p2)

            # ---- y pooling; ys tiles overlay dead y-load slots ----
            ys_tiles = {}

            def ypool(bt):
                yl = yls[bt]
                q1 = prp.tile([128, M], BF16, tag="q1", name=f"yq1_{bt}")
                q2 = prp.tile([128, M], BF16, tag="q2", name=f"yq2_{bt}")
                # gpsimd (slow) helps only on mid tiles, never the tail
                e2 = nc.gpsimd if 1 <= bt <= 4 else nc.vector
                ysb = yload.tile([128, M], BF16, tag="yl", name=f"ys{bt}")
                ys_tiles[bt] = ysb
                nc.vector.tensor_add(q1, yl[:, 0, :], yl[:, 1, :])
                e2.tensor_add(q2, yl[:, 2, :], yl[:, 3, :])
                nc.vector.tensor_add(ysb, q1, q2)

            # ---- GEMM in two 4-bt groups, kb-major ----
            or_tiles = {}
            osum = statsp.tile([128, 8], F32)

            def gemm_matmuls(bts):
                mm = {}
                for bt in bts:
                    mm[bt] = [psA.tile([128, 512], F32, tag="mmA",
                                       name=f"mmA{bt}"),
                              psB.tile([128, 512], F32, tag="mmB",
                                       name=f"mmB{bt}")]
                for kb in range(8):
                    for bt in bts:
                        for mh in range(2):
                            nc.tensor.matmul(mm[bt][mh],
                                             xts[:, kb, ts(bt, 128)],
                                             wsum_all[:, kb, ts(mh, 512)],
                                             start=(kb == 0), stop=False)
                return mm

            def gemm_close(mm, bt):
                orb = orb_r[bt]
                or_tiles[bt] = orb
                for mh in range(2):
                    nc.tensor.matmul(mm[bt][mh], ones_row_bf,
                                     bsum_bf[:, ts(mh, 512)],
                                     start=False, stop=True)
                    # drain PSUM + add pooled-y on vector (free at y-tail)
                    nc.vector.tensor_add(orb[:, ts(mh, 512)], mm[bt][mh],
                                         ys_tiles[bt][:, ts(mh, 512)])
                nc.vector.reduce_sum(out=osum[:, bt:bt + 1], in_=orb,
                                     axis=mybir.AxisListType.X)

            mmA_ = gemm_matmuls([0, 1, 2, 3])
            ypool(0)
            ypool(1)
            for bt in (0, 1):
                gemm_close(mmA_, bt)
            ypool(2)
            ypool(3)
            for bt in (2, 3):
                gemm_close(mmA_, bt)
            mmB_ = gemm_matmuls([4, 5, 6, 7])
            ypool(4)
            ypool(5)
            for bt in (4, 5):
                gemm_close(mmB_, bt)
            ypool(6)
            ypool(7)
            for bt in (6, 7):
                gemm_close(mmB_, bt)

            # ---- local sum -> global sum exchange ----
            part = statsp.tile([128, 1], F32)
            nc.vector.reduce_sum(out=part, in_=osum,
                                 axis=mybir.AxisListType.X)
            ls_ps = psA.tile([1, 1], F32, tag="mmA", name="ls_ps")
            nc.tensor.matmul(ls_ps, part, ones_col, start=True, stop=True)
            ls2 = statsp.tile([1, 1], F32)
            nc.scalar.copy(out=ls2, in_=ls_ps)

            if use_remote:
                bc_ps = psB.tile([128, 1], F32, tag="mmB", name="bc_ps")
                nc.tensor.matmul(bc_ps, ones_row_f, ls2, start=True,
                                 stop=True)
                nc.scalar.copy(out=loc128, in_=bc_ps)
                nc.vector.tensor_copy(out=slots[:, 0:1], in_=loc128)
            else:
                cc_in = dram.tile([1, 1], F32)
                cc_out = dram.tile([8, 1], F32)
                nc.sync.dma_start(out=cc_in, in_=ls2)
                # AllGather of the 8 scalars: one ring pass (vs two for
                # AllReduce); sum + partition-broadcast done locally
                nc.gpsimd.collective_compute(
                    "AllGather", mybir.AluOpType.bypass,
                    replica_groups=[list(range(N_CORES))],
                    ins=[cc_in.opt()], outs=[cc_out.opt()])
                g8 = statsp.tile([1, 8], F32)
                nc.sync.dma_start(out=g8, in_=cc_out.rearrange("a o -> o a"))
                ls3 = statsp.tile([1, 1], F32)
                nc.vector.reduce_sum(out=ls3, in_=g8,
                                     axis=mybir.AxisListType.X)
                gb_ps = psB.tile([128, 1], F32, tag="mmB", name="gb_ps")
                nc.tensor.matmul(gb_ps, ones_row_f, ls3, start=True,
                                 stop=True)
                gsb = statsp.tile([128, 1], F32)
                nc.scalar.copy(out=gsb, in_=gb_ps)
                nc.vector.reciprocal(rsb, gsb)
                for bt in range(8):
                    ot = outp.tile([128, M], BF16)
                    nc.vector.tensor_scalar(out=ot, in0=or_tiles[bt],
                                            scalar1=rsb, scalar2=TOT,
                                            op0=MULT, op1=MULT)
                    ring[bt % 2].dma_start(out=out[:, bt, :], in_=ot)

    if use_remote:
        # ---- raw region (outside Tile): XOR all-to-all scalar exchange,
        # then scales + stores with manual semaphores.  The Tile scheduler
        # cannot model remote semaphore arrivals, hence the split. ----
        ps = nc.alloc_semaphore("xch_prep")
        rs = nc.alloc_semaphore("xch_recv")
        lsm = nc.alloc_semaphore("xch_sent")
        sv = nc.alloc_semaphore("xch_scaled")
        so = nc.alloc_semaphore("xch_stored")
        for d in range(1, 8):
            rdests = [None] * 8
            rdests[d] = (0, d)
            nc.gpsimd.remote_dma_broadcast(
                out_ap=slots[:, d:d + 1], in_ap=loc128,
                remote_sem=rs, local_sem=lsm,
                rdests=rdests).then_inc(ps, 1)
        nc.gpsimd.wait_ge(ps, 7)
        nc.gpsimd.trigger_dma(count=7)
        nc.vector.wait_ge(rs, 14)
        nc.vector.reduce_sum(out=gsr, in_=slots, axis=mybir.AxisListType.X)
        nc.vector.reciprocal(rsb, gsr)
        for bt in range(8):
            nc.vector.tensor_scalar(out=orb_r[bt], in0=orb_r[bt],
                                    scalar1=rsb, scalar2=TOT,
                                    op0=MULT, op1=MULT).then_inc(sv, 1)
        for bt in range(8):
            ring[bt % 2].wait_ge(sv, bt + 1)
            ring[bt % 2].dma_start(out=out[:, bt, :],
                                   in_=orb_r[bt]).then_inc(so, 16)
        nc.gpsimd.wait_ge(lsm, 112)
        nc.gpsimd.wait_ge(so, 128)
        nc.clear_and_free_semaphores([ps, rs, lsm, sv, so])

    nc.compile()
    return nc


def _stage_x(a):
    """x slice [BL, KF] -> transpose -> bf16 [128, 8, BL]."""
    t = a.T.astype(ml_dtypes.bfloat16).reshape(8, 128, BL)
    return np.ascontiguousarray(t.transpose(1, 0, 2))


def _stage_pooled(a, perm, trans):
    """[rows, 4096] (optionally transposed first) -> bf16 [128, T, 4, M]."""
    if trans:
        a = a.T
    a = a[:, perm]                                   # (q, m) order
    r = a.shape[0]
    t = a.astype(ml_dtypes.bfloat16).reshape(r // 128, 128, 4, M)
    return np.ascontiguousarray(t.transpose(1, 0, 2, 3))


def _run(inputs, trace=False):
    if "nc" not in _CACHE:
        _CACHE["nc"] = build_nc()
    nc = _CACHE["nc"]
    x = np.asarray(inputs["x"], dtype=np.float32)
    y = np.asarray(inputs["y"], dtype=np.float32)
    w = np.asarray(inputs["weight"], dtype=np.float32)
    b = np.asarray(inputs["bias"], dtype=np.float32).reshape(1, NF)
    b = b.astype(ml_dtypes.bfloat16)
    perm = _pool_perm()
    wt_host = _stage_pooled(w, perm, trans=True)     # [128, 8, 4, 1024]
    in_maps = [
        {"xt": _stage_x(x[c * BL:(c + 1) * BL]),
         "y": _stage_pooled(y[c * BL:(c + 1) * BL], perm, trans=False),
         "wt": wt_host, "b": b}
        for c in range(N_CORES)
    ]
    res = run_bass_kernel_spmd(nc, in_maps, core_ids=list(range(N_CORES)),
                               trace=trace)
    full = np.concatenate(
        [res.results[c]["out"].transpose(1, 0, 2).reshape(BL, M)
         for c in range(N_CORES)], axis=0)
    return full.astype(np.float32).reshape(B, 1, 32, 32), res


def kernel(**inputs) -> np.ndarray:
    out, _ = _run(inputs, trace=False)
    return out
